# revision 70
# baseline (speedup 1.0000x reference)
"""MQA attention (B=2, Lq=Lkv=2048, F=1024, H=16, D=64) on 8 TRN2 cores.

Sharding: core = (batch, query-block-of-512). Each core computes its full
output rows (all 16 heads + output projection) -> no collectives; host
concatenates per-core yT slabs.

v2 (pipelined): fp16 inputs/weights, KV projection first, then an
ACT-bound software pipeline over 8 head-pairs where pair j+1's
q-projection + RoPE are emitted inside pair j's attention chunk loop.
Mask multiplies alternate DVE/GpSimd; softmax denominators use
reciprocal_approx_fast + DMA partition-broadcast off the critical path.

Per-core dataflow:
  kvT[kd|vd,lk] = Wkv.T @ xkvT          (fp16 x, fp16 w, fp32 psum)
  RoPE in halves-permuted basis (host permutes Wq/Wk columns):
  x_rot = x*cos + Swap @ (x*sin_signed), Swap = permutation on the PE.
  qT[hd,lq] per pair = Wq'.T @ xqT  (+RoPE, scaled 1/sqrt(D) via tables)
  S^T[lk,lq] per head = k-chunk.T @ qT   (fp16, zero-padded K=128)
  P^T = exp(S^T) * maskT  (ACT exp -> fp16; mask mul on DVE/GpSimd)
  O_aug^T = V_aug-chunk.T @ P^T  (ones column -> row 64 = denominator)
  normalize: reciprocal_approx_fast(Z) -> DMA broadcast -> DVE mul
  yT[f,lq] = Wo-chunks.T @ obig (+bo)
"""

import numpy as np

import concourse.bass as bass
import concourse.tile as tile
from concourse import bacc, mybir
from concourse import bass_utils
from concourse.bass import ts, broadcast_tensor_aps
from concourse.masks import make_identity

F32 = mybir.dt.float32
F16 = mybir.dt.float16

B, L, F, H, D = 2, 2048, 1024, 16, 64
LQ = 512            # query rows per core
LK = 2048           # kv rows (full)
NCORES = 8
PAIRS = H // 2      # head pairs (one qT partition block each)
FCH = F // 128      # f contraction chunks
KCH = LK // 128     # lk chunks
NL = LK // LQ       # kv column blocks

_CACHED = {}
DEBUG = False


def build_nc(debug=False):
    nc = bacc.Bacc("TRN2", target_bir_lowering=False, debug=False,
                   num_devices=NCORES)
    dt_in = [
        ("xq_t", [128, FCH, LQ], F16),         # [p, f, lq]
        ("xkv_t", [NL, 128, FCH, LQ], F16),    # [l, p, f, lq]
        ("mask_t", [128, KCH, 2, LQ], F16),    # [p, c, tt, lq] (doubled)
        ("maskn", [128, KCH, LQ], F16),        # mask as {0, -30} bias
        ("wq", [FCH, 128, FCH, 128], F16),     # [j, p, f, m]
        ("wkv", [128, FCH, 128], F16),         # [p, f, m]
        ("wo", [FCH, 128, FCH, 128], F16),     # [fb, p, j, m]
        ("bqbo", [128, 2 * FCH], F32),         # cols 0:8 bq-blocks, 8:16 bo
        ("bkv", [2 * D], F32),
        ("cosq", [128, LQ], F32),
        ("sinq", [128, LQ], F32),
        ("cksk", [D, 2 * LK], F16),            # [p, (cos|sin)*lk]
    ]
    t = {name: nc.dram_tensor(name, shape, dt, kind="ExternalInput")
         for name, shape, dt in dt_in}
    yT = nc.dram_tensor("yT", [F, LQ], F16, kind="ExternalOutput")
    dbg = {}
    if debug:
        for name, shape, dt in [
            ("d_qrot0", [128, LQ], F16), ("d_ktop", [128, LK], F16),
            ("d_kvraw", [128, LK], F32), ("d_pt0", [128, 2, LQ], F16),
            ("d_pt15", [128, 2, LQ], F16), ("d_oa0", [128, LQ], F32),
            ("d_ob0", [128, LQ], F32),
            ("d_obig", [128, PAIRS, LQ], F16), ("d_vaug", [128, KCH, D + 1], F16),
        ]:
            dbg[name] = nc.dram_tensor(name, shape, dt, kind="ExternalOutput")

    with tile.TileContext(nc) as tc:
        with (
            tc.tile_pool(name="persist", bufs=1) as persist,
            tc.tile_pool(name="ptiles", bufs=4) as ptp,
            tc.tile_pool(name="small", bufs=1) as small,
            tc.tile_pool(name="xin", bufs=4) as xin,
            tc.tile_pool(name="wst", bufs=3) as wst,
            tc.tile_pool(name="ktmp", bufs=1) as ktmp,
            tc.tile_pool(name="rtp", bufs=2) as rtp,
            tc.tile_pool(name="qpool", bufs=2) as qpool,
            tc.tile_pool(name="recp", bufs=2) as recp,
            tc.tile_pool(name="yout", bufs=2) as yout,
            tc.tile_pool(name="psst", bufs=2, space="PSUM") as psst,
            tc.tile_pool(name="psacc", bufs=2, space="PSUM") as psacc,
            tc.tile_pool(name="pssm", bufs=2, space="PSUM") as pssm,
        ):
            # ---- input DMAs (priority order) ----
            wkv_sb = persist.tile([128, FCH, 128], F16)
            nc.sync.dma_start(wkv_sb, t["wkv"].ap())

            # split transfers into ~256KB pieces with >=2KB lines so they
            # round-robin across the 16 DMA engines in parallel
            mt = persist.tile([128, KCH, 2, LQ], F16)    # maskT resident
            for c in range(KCH):
                nc.gpsimd.dma_start(mt[:, c, :, :], t["mask_t"].ap()[:, c])
            mtn = persist.tile([128, KCH, LQ], F16)      # mask bias {0,-30}
            for i in range(KCH // 2):
                nc.gpsimd.dma_start(mtn[:, 2 * i:2 * i + 2, :],
                                    t["maskn"].ap()[:, 2 * i:2 * i + 2, :])

            # xkv blocks 0/1 first (sync/scalar) so kv-proj starts early;
            # xq + tables land on scalar between blocks 1 and 3
            xin_tiles = {}
            for l in (0, 1):
                xkv_e = xin.tile([128, FCH, LQ], F16, tag="x",
                                 name=f"xkv{l}")
                eng = nc.sync if l % 2 == 0 else nc.scalar
                for i in range(FCH // 2):
                    eng.dma_start(xkv_e[:, 2 * i:2 * i + 2, :],
                                  t["xkv_t"].ap()[l][:, 2 * i:2 * i + 2, :])
                xin_tiles[l] = xkv_e

            xq = persist.tile([128, FCH, LQ], F16)
            for i in range(FCH // 2):
                nc.scalar.dma_start(xq[:, 2 * i:2 * i + 2, :],
                                    t["xq_t"].ap()[:, 2 * i:2 * i + 2, :])

            cq = persist.tile([128, LQ], F32)
            sq = persist.tile([128, LQ], F32)
            cksk = persist.tile([D, 2, LK], F16)
            nc.scalar.dma_start(cq, t["cosq"].ap())
            nc.scalar.dma_start(sq, t["sinq"].ap())
            nc.scalar.dma_start(cksk,
                                t["cksk"].ap().rearrange("p (a l) -> p a l",
                                                         a=2))
            ck = cksk[:, 0, :]
            sk = cksk[:, 1, :]

            bqbo = small.tile([128, 2 * FCH], F32, tag="bias")
            nc.scalar.dma_start(bqbo, t["bqbo"].ap())
            bq_sb = bqbo[:, 0:FCH]
            bo_sb = bqbo[:, FCH:2 * FCH]
            bkv_sb = small.tile([128, 1], F32, tag="bias2")
            nc.scalar.dma_start(bkv_sb, t["bkv"].ap().unsqueeze(1))

            wq_tiles = {}

            def prefetch_wq(j):
                wq_tiles[j] = wst.tile([128, FCH, 128], F16, tag="wq",
                                       name=f"wq_sb{j}")
                nc.sync.dma_start(wq_tiles[j], t["wq"].ap()[j])

            prefetch_wq(0)

            ones16 = small.tile([128, D], F16, tag="ones")
            nc.gpsimd.memset(ones16, 1.0)

            idt = small.tile([128, 128], F32, tag="ident")
            make_identity(nc, idt)
            idt16 = small.tile([128, 128], F16, tag="ident16")
            nc.gpsimd.tensor_copy(idt16, idt)
            # halves-swap permutation matrix: M[p, p-xor-32-within-head] = 1
            swp = small.tile([128, 128], F16, tag="swp")
            nc.gpsimd.memset(swp, 0.0)
            for o1, o2 in ((0, 32), (32, 0), (64, 96), (96, 64)):
                nc.gpsimd.affine_select(
                    out=swp[o1:o1 + 32, o2:o2 + 32],
                    in_=swp[o1:o1 + 32, o2:o2 + 32],
                    compare_op=mybir.AluOpType.not_equal, fill=1.0,
                    base=0, pattern=[[-1, 32]], channel_multiplier=1)

            # persistent SBUF state
            qrot = {}                                     # per-pair tiles
            ktop = persist.tile([128, LK], F16)           # k in rows 0:64
            kbot = persist.tile([128, LK], F16)           # k in rows 64:128
            vaug = persist.tile([128, KCH, D + 1], F16)   # V chunks + ones
            obig = persist.tile([128, PAIRS, LQ], F16)    # normalized O^T
            wo_sb = persist.tile([128, FCH, FCH, 128], F16)  # [p, fb, j, m]

            # ================= per-pair q-proj + RoPE =================
            qp_state = {}

            def emit_qproj_slice(j, f0, f1):
                # partial contraction f0:f1 of the q projection; lets the
                # matmuls interleave with attention chunks so ACT stays fed
                if f0 == 0:
                    qp_state[j, "psq"] = pssm.tile([128, LQ], F32, tag="sm",
                                                   name=f"psq{j}")
                    qp_state[j, "wq"] = wq_tiles.pop(j)
                psq = qp_state[j, "psq"]
                wq_j = qp_state[j, "wq"]
                for f in range(f0, f1):
                    nc.tensor.matmul(psq, wq_j[:, f, :], xq[:, f, :],
                                     start=(f == 0), stop=(f == FCH - 1))

            def emit_qproj_bias(j):
                psq = qp_state.pop((j, "psq"))
                qp_state.pop((j, "wq"))
                # tmq = (psq + bq) * sin ; qc = (psq + bq) * cos
                tmq = rtp.tile([128, LQ], F16, tag="qsin")
                nc.vector.scalar_tensor_tensor(
                    out=tmq, in0=psq, scalar=bq_sb[:, j:j + 1], in1=sq,
                    op0=mybir.AluOpType.add, op1=mybir.AluOpType.mult)
                qc = rtp.tile([128, LQ], F32, tag="qcos")
                nc.vector.scalar_tensor_tensor(
                    out=qc, in0=psq, scalar=bq_sb[:, j:j + 1], in1=cq,
                    op0=mybir.AluOpType.add, op1=mybir.AluOpType.mult)
                qp_state[j] = (tmq, qc)

            def emit_rope_finish(j):
                tmq, qc = qp_state.pop(j)
                psw = pssm.tile([128, LQ], F32, tag="sm")
                nc.tensor.matmul(psw, swp, tmq, start=True, stop=True)
                qrot[j] = qpool.tile([128, LQ], F16, tag="qrot",
                                     name=f"qrot{j}")
                nc.vector.tensor_add(qrot[j], qc, psw)

            def emit_qproj_rope(j):
                emit_qproj_slice(j, 0, FCH)
                emit_qproj_bias(j)
                emit_rope_finish(j)

            # ================= phase KV: projection + RoPE =================
            # prefetch ALL xkv blocks first so the 16 transfers spread
            # across the DMA engines in parallel
            kvraw = persist.tile([128, LK], F32)
            nc.vector.memset(vaug[:, :, D:D + 1], 1.0)
            nc.vector.memset(ktop[64:128], 0.0)
            nc.vector.memset(kbot[0:64], 0.0)
            xkv_tiles = []
            for l in range(NL):
                if l in xin_tiles:
                    xkv_tiles.append(xin_tiles[l])
                    continue
                xkv = xin.tile([128, FCH, LQ], F16, tag="x",
                               name=f"xkv{l}")
                eng = nc.sync if l % 2 == 0 else nc.scalar
                for i in range(FCH // 2):
                    eng.dma_start(xkv[:, 2 * i:2 * i + 2, :],
                                  t["xkv_t"].ap()[l][:, 2 * i:2 * i + 2, :])
                xkv_tiles.append(xkv)
            # per l-block: project, bias, RoPE, kbot copy, vaug transpose
            for l in range(NL):
                xkv = xkv_tiles[l]
                pkv = pssm.tile([128, LQ], F32, tag="sm")
                for f in range(FCH):
                    nc.tensor.matmul(pkv, wkv_sb[:, f, :], xkv[:, f, :],
                                     start=(f == 0), stop=(f == FCH - 1))
                nc.vector.tensor_scalar_add(kvraw[:, ts(l, LQ)], pkv,
                                            bkv_sb[:, 0:1])
                tmk = ktmp.tile([D, LQ], F16, tag="ksin")
                nc.vector.tensor_mul(tmk, kvraw[0:64, ts(l, LQ)],
                                     sk[:, ts(l, LQ)])
                kc = ktmp.tile([D, LQ], F16, tag="kcos")
                nc.vector.tensor_mul(kc, kvraw[0:64, ts(l, LQ)],
                                     ck[:, ts(l, LQ)])
                pswk = pssm.tile([128, LQ], F32, tag="sm")
                nc.tensor.matmul(pswk[0:64], swp[0:64, 0:64], tmk,
                                 start=True, stop=True)
                nc.vector.tensor_add(ktop[0:64, ts(l, LQ)], kc, pswk[0:64])
                nc.gpsimd.dma_start(kbot[64:128, ts(l, LQ)],
                                    ktop[0:64, ts(l, LQ)])
                for cc in range(NL):
                    c = l * NL + cc
                    tp = pssm.tile([128, LQ], F32, tag="sm")
                    nc.tensor.transpose(tp[:, 0:64],
                                        kvraw[64:128, ts(c, 128)],
                                        idt[64:128, 64:128])
                    nc.vector.tensor_copy(vaug[:, c, 0:D], tp[:, 0:64])
                if l == 0:
                    emit_qproj_rope(0)
                    prefetch_wq(1)

            if debug:
                nc.sync.dma_start(dbg["d_qrot0"].ap(), qrot[0])

            # ================= attention pair loop =================
            # chunks where the mask is folded into S on the PE (exp of
            # S-30 flushes to 0 in fp16); a couple on GpSimd; rest DVE
            PE_SET = (1, 4, 6, 11, 13)
            GP_SET = ()
            norm_state = {}
            # persistent transposed-recip scratch; cols != 64 stay zero
            rz = [persist.tile([128, NL, 128], F16, name=f"rz{tt}")
                  for tt in range(2)]
            nc.gpsimd.memset(rz[0], 0.0)
            nc.gpsimd.memset(rz[1], 0.0)

            def emit_evacuate(j, oa, ob):
                # copy O_unnorm + Z row out of PSUM; frees the banks
                for tt, op in ((0, oa), (1, ob)):
                    osb = recp.tile([128, LQ], F16, tag=f"osb{tt}",
                                    name=f"osb{tt}_{j}")
                    nc.vector.tensor_copy(osb[0:D + 1, :], op[0:D + 1, :])
                    norm_state[(j, tt)] = osb

            def emit_recip(j, tt):
                # transpose Z (row 64) so lq sits on partitions, then a
                # free-size-4 reciprocal, then transpose back
                osb = norm_state[(j, tt)]
                ztp = pssm.tile([128, NL, 128], F16, tag="sm",
                                name=f"ztp{tt}_{j}")
                for b in range(NL):
                    nc.tensor.transpose(ztp[:, b, :], osb[:, ts(b, 128)],
                                        idt16)
                with nc.allow_low_precision(reason="fp16 softmax recip"):
                    nc.vector.reciprocal(rz[tt][:, :, 64:65],
                                         ztp[:, :, 64:65])
                rbt = pssm.tile([128, NL, 128], F16, tag="sm",
                                name=f"rbt{tt}_{j}")
                for b in range(NL):
                    nc.tensor.transpose(rbt[:, b, :], rz[tt][:, b, :], idt16)
                recs = recp.tile([65, NL, 128], F16, tag=f"rec{tt}",
                                 name=f"recs{tt}_{j}")
                nc.vector.tensor_copy(recs[64:65, :, :], rbt[64:65, :, :])
                norm_state[(j, tt, "rec")] = recs

            def emit_norm_finish(j, tt):
                osb = norm_state.pop((j, tt))
                recs = norm_state.pop((j, tt, "rec"))
                rbp = pssm.tile([128, LQ], F32, tag="sm")
                nc.tensor.matmul(rbp[0:D, :], ones16[D:D + 1, :],
                                 recs[64:65, :, :], start=True, stop=True)
                rbs = recp.tile([D, LQ], F16, tag="rbs")
                nc.vector.tensor_copy(rbs, rbp[0:D, :])
                if tt == 0:
                    nc.vector.tensor_mul(obig[0:D, j, :], osb[0:D, :], rbs)
                else:
                    ofin = recp.tile([D, LQ], F16, tag="ofin")
                    nc.vector.tensor_mul(ofin, osb[0:D, :], rbs)
                    nc.gpsimd.dma_start(obig[64:128, j, :], ofin)

            for j in range(PAIRS):
                oa = psacc.tile([128, LQ], F32, tag="acc")
                ob = psacc.tile([128, LQ], F32, tag="acc")

                def emit_pv(c, pt):
                    nc.tensor.matmul(oa[0:D + 1, :], vaug[:, c, :],
                                     pt[:, 0, :], start=(c == 0),
                                     stop=(c == KCH - 1))
                    nc.tensor.matmul(ob[0:D + 1, :], vaug[:, c, :],
                                     pt[:, 1, :], start=(c == 0),
                                     stop=(c == KCH - 1))

                prev = None
                for c in range(KCH):
                    pe_mask = c in PE_SET
                    st = psst.tile([128, 2, LQ], F32, tag="st")
                    for tt, kt in ((0, ktop), (1, kbot)):
                        nc.tensor.matmul(st[:, tt, :], kt[:, ts(c, 128)],
                                         qrot[j], start=True,
                                         stop=not pe_mask)
                        if pe_mask:
                            nc.tensor.matmul(st[:, tt, :], idt16,
                                             mtn[:, c, :], start=False,
                                             stop=True)
                    pt = ptp.tile([128, 2, LQ], F16, tag="p")
                    nc.scalar.activation(pt, st,
                                         mybir.ActivationFunctionType.Exp)
                    if not pe_mask:
                        eng = nc.gpsimd if c in GP_SET else nc.vector
                        eng.tensor_tensor(out=pt, in0=pt,
                                          in1=mt[:, c, :, :],
                                          op=mybir.AluOpType.mult)
                    if debug and j == 0 and c == 0:
                        nc.sync.dma_start(dbg["d_pt0"].ap(), pt)
                    if debug and j == 0 and c == KCH - 1:
                        nc.sync.dma_start(dbg["d_pt15"].ap(), pt)
                    # PV lags one chunk so the PE never waits on exp+mask
                    if prev is not None:
                        emit_pv(c - 1, prev)
                    prev = pt
                    if j > 0:
                        if c == 2:
                            emit_recip(j - 1, 0)
                        elif c == 7:
                            emit_recip(j - 1, 1)
                        elif c == 8:
                            emit_norm_finish(j - 1, 0)
                        elif c == 12:
                            emit_norm_finish(j - 1, 1)
                    if c == 3 and j + 1 < PAIRS:
                        emit_qproj_slice(j + 1, 0, 3)
                    elif c == 4 and j + 1 < PAIRS:
                        emit_qproj_slice(j + 1, 3, 6)
                    elif c == 5 and j + 1 < PAIRS:
                        emit_qproj_slice(j + 1, 6, FCH)
                        emit_qproj_bias(j + 1)
                    elif c == 6 and j + 1 < PAIRS:
                        emit_rope_finish(j + 1)
                    elif c == 13 and j + 2 < PAIRS:
                        prefetch_wq(j + 2)
                    elif c == 14 and j < FCH:
                        # prefetch one wo block per pair into wo_sb
                        nc.sync.dma_start(wo_sb[:, j, :, :], t["wo"].ap()[j])
                emit_pv(KCH - 1, prev)
                if debug and j == 0:
                    dcp_a = yout.tile([128, LQ], F32, tag="y")
                    nc.vector.tensor_copy(dcp_a, oa)
                    nc.sync.dma_start(dbg["d_oa0"].ap(), dcp_a)
                    dcp_b = yout.tile([128, LQ], F32, tag="y")
                    nc.vector.tensor_copy(dcp_b, ob)
                    nc.sync.dma_start(dbg["d_ob0"].ap(), dcp_b)
                emit_evacuate(j, oa, ob)

            # finish last pair's normalization
            emit_recip(PAIRS - 1, 0)
            emit_recip(PAIRS - 1, 1)
            emit_norm_finish(PAIRS - 1, 0)
            emit_norm_finish(PAIRS - 1, 1)

            if debug:
                nc.sync.dma_start(dbg["d_ktop"].ap(), ktop)
                nc.sync.dma_start(dbg["d_kvraw"].ap(), kvraw)
                nc.sync.dma_start(dbg["d_obig"].ap(), obig)
                nc.sync.dma_start(dbg["d_vaug"].ap(), vaug)

            # ================= phase D: output projection =================
            for fb in range(FCH):
                psy = psacc.tile([128, LQ], F32, tag="acc")
                for j in range(FCH):
                    nc.tensor.matmul(psy, wo_sb[:, fb, j, :], obig[:, j, :],
                                     start=(j == 0), stop=(j == FCH - 1))
                ysb = yout.tile([128, LQ], F16, tag="y")
                nc.vector.tensor_scalar_add(ysb, psy, bo_sb[:, fb:fb + 1])
                out_eng = (nc.sync, nc.scalar, nc.gpsimd)[fb % 3]
                out_eng.dma_start(yT.ap()[ts(fb, 128), :], ysb)

    nc.compile()
    return nc


def _tables():
    """RoPE tables in halves-permuted basis: rows i (even-half) hold +sin,
    rows 32+i (odd-half) hold -sin (for the tmp-then-swap formulation)."""
    inv_freq = 1.0 / (10000.0 ** (np.arange(0, D, 2, dtype=np.float64) / D))
    ang = np.outer(inv_freq, np.arange(L, dtype=np.float64))  # [32, L]
    cos = np.cos(ang).astype(np.float32)
    sin = np.sin(ang).astype(np.float32)
    cos64 = np.concatenate([cos, cos], axis=0)                # [64, L]
    sin_sgn = np.concatenate([sin, -sin], axis=0)             # [64, L]
    return cos64, sin_sgn


def _prep_weights(Wq, bq, Wk, bk, Wv, bv, Wo, bo):
    perm = np.concatenate([np.arange(0, D, 2), np.arange(1, D, 2)])
    WqP = np.asarray(Wq, dtype=np.float32)[:, :, perm].reshape(F, H * D)
    bqP = np.asarray(bq, dtype=np.float32)[:, perm].reshape(H * D)
    WkP = np.asarray(Wk, dtype=np.float32)[:, perm]
    bkP = np.asarray(bk, dtype=np.float32)[perm]
    Wkv = np.concatenate([WkP, np.asarray(Wv, dtype=np.float32)], axis=1)
    bkv = np.concatenate([bkP, np.asarray(bv, dtype=np.float32)])
    WoR = np.asarray(Wo, dtype=np.float32).reshape(H * D, F)
    bo_ = np.asarray(bo, dtype=np.float32)

    wq_pre = np.ascontiguousarray(
        WqP.reshape(FCH, 128, FCH, 128).transpose(2, 1, 0, 3)
    ).astype(np.float16)
    wkv_pre = np.ascontiguousarray(
        Wkv.reshape(FCH, 128, 128).transpose(1, 0, 2)).astype(np.float16)
    wo_pre = np.ascontiguousarray(
        WoR.reshape(FCH, 128, FCH, 128).transpose(2, 1, 0, 3)
    ).astype(np.float16)
    bqbo = np.ascontiguousarray(np.concatenate(
        [bqP.reshape(FCH, 128).T, bo_.reshape(FCH, 128).T], axis=1))
    return wq_pre, wkv_pre, wo_pre, bqbo, bkv


def kernel(inputs_q, inputs_kv, mask, Wq, bq, Wk, bk, Wv, bv, Wo, bo):
    if "nc" not in _CACHED:
        _CACHED["nc"] = build_nc(debug=DEBUG)
    nc = _CACHED["nc"]

    wq_pre, wkv_pre, wo_pre, bqbo, bkv = _prep_weights(
        Wq, bq, Wk, bk, Wv, bv, Wo, bo)

    cos64, sin_sgn = _tables()
    scale = 1.0 / np.sqrt(np.float32(D))
    cksk = np.ascontiguousarray(
        np.concatenate([cos64, sin_sgn], axis=1)).astype(np.float16)
    cosq_full = np.tile(cos64 * scale, (2, 1))         # [128, L]
    sinq_full = np.tile(sin_sgn * scale, (2, 1))

    xq = np.asarray(inputs_q, dtype=np.float32)
    xkv = np.asarray(inputs_kv, dtype=np.float32)
    mk = np.asarray(mask)

    in_maps = []
    for core in range(NCORES):
        b = core // 4
        qs = (core % 4) * LQ
        xq_t = np.ascontiguousarray(
            xq[b, qs:qs + LQ, :].T.reshape(FCH, 128, LQ).transpose(1, 0, 2)
        ).astype(np.float16)
        xkv_t = np.ascontiguousarray(
            xkv[b].T.reshape(FCH, 128, NL, LQ).transpose(2, 1, 0, 3)
        ).astype(np.float16)
        mask_1 = mk[b, 0, qs:qs + LQ, :].T.reshape(KCH, 128, LQ)
        mask_t = np.ascontiguousarray(np.broadcast_to(
            mask_1[:, :, None, :], (KCH, 128, 2, LQ)).transpose(1, 0, 2, 3)
        ).astype(np.float16)
        maskn = np.ascontiguousarray(
            ((mask_1.astype(np.float32) - 1.0) * 30.0).transpose(1, 0, 2)
        ).astype(np.float16)
        in_maps.append({
            "xq_t": xq_t,
            "xkv_t": xkv_t,
            "mask_t": mask_t,
            "maskn": maskn,
            "wq": wq_pre,
            "wkv": wkv_pre,
            "wo": wo_pre,
            "bqbo": bqbo,
            "bkv": bkv,
            "cosq": np.ascontiguousarray(cosq_full[:, qs:qs + LQ]),
            "sinq": np.ascontiguousarray(sinq_full[:, qs:qs + LQ]),
            "cksk": cksk,
        })

    res = bass_utils.run_bass_kernel_spmd(nc, in_maps,
                                          core_ids=list(range(NCORES)))
    _CACHED["last_results"] = res
    _CACHED["last_maps"] = in_maps

    out = np.empty((B, L, F), dtype=np.float32)
    for core in range(NCORES):
        b = core // 4
        qs = (core % 4) * LQ
        out[b, qs:qs + LQ, :] = res.results[core]["yT"].T.astype(np.float32)
    return out


# revision 74
# speedup vs baseline: 1.0332x; 1.0332x over previous
"""MQA attention (B=2, Lq=Lkv=2048, F=1024, H=16, D=64) on 8 TRN2 cores.

Sharding: core = (batch, query-block-of-512). Each core computes its full
output rows (all 16 heads + output projection) -> no collectives; host
concatenates per-core yT slabs.

v2 (pipelined): fp16 inputs/weights, KV projection first, then an
ACT-bound software pipeline over 8 head-pairs where pair j+1's
q-projection + RoPE are emitted inside pair j's attention chunk loop.
Mask multiplies alternate DVE/GpSimd; softmax denominators use
reciprocal_approx_fast + DMA partition-broadcast off the critical path.

Per-core dataflow:
  kvT[kd|vd,lk] = Wkv.T @ xkvT          (fp16 x, fp16 w, fp32 psum)
  RoPE in halves-permuted basis (host permutes Wq/Wk columns):
  x_rot = x*cos + Swap @ (x*sin_signed), Swap = permutation on the PE.
  qT[hd,lq] per pair = Wq'.T @ xqT  (+RoPE, scaled 1/sqrt(D) via tables)
  S^T[lk,lq] per head = k-chunk.T @ qT   (fp16, zero-padded K=128)
  P^T = exp(S^T) * maskT  (ACT exp -> fp16; mask mul on DVE/GpSimd)
  O_aug^T = V_aug-chunk.T @ P^T  (ones column -> row 64 = denominator)
  normalize: reciprocal_approx_fast(Z) -> DMA broadcast -> DVE mul
  yT[f,lq] = Wo-chunks.T @ obig (+bo)
"""

import numpy as np

import concourse.bass as bass
import concourse.tile as tile
from concourse import bacc, mybir
from concourse import bass_utils
from concourse.bass import ts, broadcast_tensor_aps
from concourse.masks import make_identity

F32 = mybir.dt.float32
F16 = mybir.dt.float16

B, L, F, H, D = 2, 2048, 1024, 16, 64
LQ = 512            # query rows per core
LK = 2048           # kv rows (full)
NCORES = 8
PAIRS = H // 2      # head pairs (one qT partition block each)
FCH = F // 128      # f contraction chunks
KCH = LK // 128     # lk chunks
NL = LK // LQ       # kv column blocks

_CACHED = {}
DEBUG = False


def build_nc(debug=False):
    nc = bacc.Bacc("TRN2", target_bir_lowering=False, debug=False,
                   num_devices=NCORES)
    dt_in = [
        ("xq_t", [128, FCH, LQ], F16),         # [p, f, lq]
        ("xkv_t", [NL, 128, FCH, LQ], F16),    # [l, p, f, lq]
        ("mask_t", [128, KCH, 2, LQ], F16),    # [p, c, tt, lq] (doubled)
        ("maskn", [128, KCH, LQ], F16),        # mask as {0, -30} bias
        ("wq", [FCH, 128, FCH, 128], F16),     # [j, p, f, m]
        ("wkv", [128, FCH, 128], F16),         # [p, f, m]
        ("wo", [FCH, 128, FCH, 128], F16),     # [fb, p, j, m]
        ("bqbo", [128, 2 * FCH], F32),         # cols 0:8 bq-blocks, 8:16 bo
        ("bkv", [2 * D], F32),
        ("cosq", [128, LQ], F32),
        ("sinq", [128, LQ], F32),
        ("cksk", [D, 2 * LK], F16),            # [p, (cos|sin)*lk]
    ]
    t = {name: nc.dram_tensor(name, shape, dt, kind="ExternalInput")
         for name, shape, dt in dt_in}
    yT = nc.dram_tensor("yT", [F, LQ], F16, kind="ExternalOutput")
    dbg = {}
    if debug:
        for name, shape, dt in [
            ("d_qrot0", [128, LQ], F16), ("d_ktop", [128, LK], F16),
            ("d_kvraw", [128, LK], F32), ("d_pt0", [128, 2, LQ], F16),
            ("d_pt15", [128, 2, LQ], F16), ("d_oa0", [128, LQ], F32),
            ("d_ob0", [128, LQ], F32),
            ("d_obig", [128, PAIRS, LQ], F16), ("d_vaug", [128, KCH, D + 1], F16),
        ]:
            dbg[name] = nc.dram_tensor(name, shape, dt, kind="ExternalOutput")

    with tile.TileContext(nc) as tc:
        with (
            tc.tile_pool(name="persist", bufs=1) as persist,
            tc.tile_pool(name="ptiles", bufs=6) as ptp,
            tc.tile_pool(name="small", bufs=1) as small,
            tc.tile_pool(name="xin", bufs=4) as xin,
            tc.tile_pool(name="wst", bufs=3) as wst,
            tc.tile_pool(name="ktmp", bufs=1) as ktmp,
            tc.tile_pool(name="rtp", bufs=3) as rtp,
            tc.tile_pool(name="qpool", bufs=2) as qpool,
            tc.tile_pool(name="recp", bufs=2) as recp,
            tc.tile_pool(name="yout", bufs=2) as yout,
            tc.tile_pool(name="psst", bufs=2, space="PSUM") as psst,
            tc.tile_pool(name="psacc", bufs=2, space="PSUM") as psacc,
            tc.tile_pool(name="pssm", bufs=2, space="PSUM") as pssm,
        ):
            # ---- input DMAs (priority order) ----
            wkv_sb = persist.tile([128, FCH, 128], F16)
            nc.sync.dma_start(wkv_sb, t["wkv"].ap())

            # split transfers into ~256KB pieces with >=2KB lines so they
            # round-robin across the 16 DMA engines in parallel
            mt = persist.tile([128, KCH, 2, LQ], F16)    # maskT resident
            for c in range(KCH):
                nc.gpsimd.dma_start(mt[:, c, :, :], t["mask_t"].ap()[:, c])
            mtn = persist.tile([128, KCH, LQ], F16)      # mask bias {0,-30}
            for i in range(KCH // 2):
                nc.gpsimd.dma_start(mtn[:, 2 * i:2 * i + 2, :],
                                    t["maskn"].ap()[:, 2 * i:2 * i + 2, :])

            xq = persist.tile([128, FCH, LQ], F16)
            for i in range(FCH // 2):
                nc.scalar.dma_start(xq[:, 2 * i:2 * i + 2, :],
                                    t["xq_t"].ap()[:, 2 * i:2 * i + 2, :])

            cq = persist.tile([128, LQ], F32)
            sq = persist.tile([128, LQ], F32)
            cksk = persist.tile([D, 2, LK], F16)
            nc.scalar.dma_start(cq, t["cosq"].ap())
            nc.scalar.dma_start(sq, t["sinq"].ap())
            nc.scalar.dma_start(cksk,
                                t["cksk"].ap().rearrange("p (a l) -> p a l",
                                                         a=2))
            ck = cksk[:, 0, :]
            sk = cksk[:, 1, :]

            bqbo = small.tile([128, 2 * FCH], F32, tag="bias")
            nc.scalar.dma_start(bqbo, t["bqbo"].ap())
            bq_sb = bqbo[:, 0:FCH]
            bo_sb = bqbo[:, FCH:2 * FCH]
            bkv_sb = small.tile([128, 1], F32, tag="bias2")
            nc.scalar.dma_start(bkv_sb, t["bkv"].ap().unsqueeze(1))

            wq_tiles = {}

            def prefetch_wq(j):
                wq_tiles[j] = wst.tile([128, FCH, 128], F16, tag="wq",
                                       name=f"wq_sb{j}")
                nc.sync.dma_start(wq_tiles[j], t["wq"].ap()[j])

            prefetch_wq(0)

            ones16 = small.tile([128, D], F16, tag="ones")
            nc.gpsimd.memset(ones16, 1.0)

            idt = small.tile([128, 128], F32, tag="ident")
            make_identity(nc, idt)
            idt16 = small.tile([128, 128], F16, tag="ident16")
            nc.gpsimd.tensor_copy(idt16, idt)
            # halves-swap permutation matrix: M[p, p-xor-32-within-head] = 1
            swp = small.tile([128, 128], F16, tag="swp")
            nc.gpsimd.memset(swp, 0.0)
            for o1, o2 in ((0, 32), (32, 0), (64, 96), (96, 64)):
                nc.gpsimd.affine_select(
                    out=swp[o1:o1 + 32, o2:o2 + 32],
                    in_=swp[o1:o1 + 32, o2:o2 + 32],
                    compare_op=mybir.AluOpType.not_equal, fill=1.0,
                    base=0, pattern=[[-1, 32]], channel_multiplier=1)

            # persistent SBUF state
            qrot = {}                                     # per-pair tiles
            ktop = persist.tile([128, LK], F16)           # k in rows 0:64
            kbot = persist.tile([128, LK], F16)           # k in rows 64:128
            vaug = persist.tile([128, KCH, D + 1], F16)   # V chunks + ones
            obig = persist.tile([128, PAIRS, LQ], F16)    # normalized O^T
            wo_sb = persist.tile([128, FCH, FCH, 128], F16)  # [p, fb, j, m]

            # ================= per-pair q-proj + RoPE =================
            qp_state = {}

            def emit_qproj_slice(j, f0, f1):
                # partial contraction f0:f1 of the q projection; lets the
                # matmuls interleave with attention chunks so ACT stays fed
                if f0 == 0:
                    qp_state[j, "psq"] = pssm.tile([128, LQ], F32, tag="sm",
                                                   name=f"psq{j}")
                    qp_state[j, "wq"] = wq_tiles.pop(j)
                psq = qp_state[j, "psq"]
                wq_j = qp_state[j, "wq"]
                for f in range(f0, f1):
                    nc.tensor.matmul(psq, wq_j[:, f, :], xq[:, f, :],
                                     start=(f == 0), stop=(f == FCH - 1))

            def emit_qproj_bias(j):
                psq = qp_state.pop((j, "psq"))
                qp_state.pop((j, "wq"))
                # tmq = (psq + bq) * sin ; qc = (psq + bq) * cos
                tmq = rtp.tile([128, LQ], F16, tag="qsin")
                nc.vector.scalar_tensor_tensor(
                    out=tmq, in0=psq, scalar=bq_sb[:, j:j + 1], in1=sq,
                    op0=mybir.AluOpType.add, op1=mybir.AluOpType.mult)
                qc = rtp.tile([128, LQ], F32, tag="qcos")
                nc.vector.scalar_tensor_tensor(
                    out=qc, in0=psq, scalar=bq_sb[:, j:j + 1], in1=cq,
                    op0=mybir.AluOpType.add, op1=mybir.AluOpType.mult)
                qp_state[j] = (tmq, qc)

            def emit_rope_finish(j):
                tmq, qc = qp_state.pop(j)
                psw = pssm.tile([128, LQ], F32, tag="sm")
                nc.tensor.matmul(psw, swp, tmq, start=True, stop=True)
                qrot[j] = qpool.tile([128, LQ], F16, tag="qrot",
                                     name=f"qrot{j}")
                nc.vector.tensor_add(qrot[j], qc, psw)

            def emit_qproj_rope(j):
                emit_qproj_slice(j, 0, FCH)
                emit_qproj_bias(j)
                emit_rope_finish(j)

            # ================= phase KV: projection + RoPE =================
            # prefetch ALL xkv blocks first so the 16 transfers spread
            # across the DMA engines in parallel
            kvraw = persist.tile([128, LK], F32)
            nc.vector.memset(vaug[:, :, D:D + 1], 1.0)
            nc.vector.memset(ktop[64:128], 0.0)
            nc.vector.memset(kbot[0:64], 0.0)
            xkv_tiles = []
            for l in range(NL):
                xkv = xin.tile([128, FCH, LQ], F16, tag="x",
                               name=f"xkv{l}")
                for i in range(FCH // 2):
                    nc.sync.dma_start(xkv[:, 2 * i:2 * i + 2, :],
                                      t["xkv_t"].ap()[l][:, 2 * i:2 * i + 2, :])
                xkv_tiles.append(xkv)
            # per l-block: project, bias, RoPE, kbot copy, vaug transpose
            for l in range(NL):
                xkv = xkv_tiles[l]
                pkv = pssm.tile([128, LQ], F32, tag="sm")
                for f in range(FCH):
                    nc.tensor.matmul(pkv, wkv_sb[:, f, :], xkv[:, f, :],
                                     start=(f == 0), stop=(f == FCH - 1))
                nc.vector.tensor_scalar_add(kvraw[:, ts(l, LQ)], pkv,
                                            bkv_sb[:, 0:1])
                tmk = ktmp.tile([D, LQ], F16, tag="ksin")
                nc.vector.tensor_mul(tmk, kvraw[0:64, ts(l, LQ)],
                                     sk[:, ts(l, LQ)])
                kc = ktmp.tile([D, LQ], F16, tag="kcos")
                nc.vector.tensor_mul(kc, kvraw[0:64, ts(l, LQ)],
                                     ck[:, ts(l, LQ)])
                pswk = pssm.tile([128, LQ], F32, tag="sm")
                nc.tensor.matmul(pswk[0:64], swp[0:64, 0:64], tmk,
                                 start=True, stop=True)
                nc.vector.tensor_add(ktop[0:64, ts(l, LQ)], kc, pswk[0:64])
                nc.gpsimd.dma_start(kbot[64:128, ts(l, LQ)],
                                    ktop[0:64, ts(l, LQ)])
                for cc in range(NL):
                    c = l * NL + cc
                    tp = pssm.tile([128, LQ], F32, tag="sm")
                    nc.tensor.transpose(tp[:, 0:64],
                                        kvraw[64:128, ts(c, 128)],
                                        idt[64:128, 64:128])
                    nc.vector.tensor_copy(vaug[:, c, 0:D], tp[:, 0:64])
                if l == 0:
                    emit_qproj_rope(0)
                    prefetch_wq(1)

            if debug:
                nc.sync.dma_start(dbg["d_qrot0"].ap(), qrot[0])

            # ================= attention pair loop =================
            # chunks where the mask is folded into S on the PE (exp of
            # S-30 flushes to 0 in fp16); a couple on GpSimd; rest DVE
            PE_SET = (1, 4, 6, 11, 13)
            GP_SET = ()
            norm_state = {}
            # persistent transposed-recip scratch; cols != 64 stay zero
            rz = [persist.tile([128, NL, 128], F16, name=f"rz{tt}")
                  for tt in range(2)]
            nc.gpsimd.memset(rz[0], 0.0)
            nc.gpsimd.memset(rz[1], 0.0)

            def emit_evacuate(j, oa, ob):
                # copy O_unnorm + Z row out of PSUM; frees the banks
                for tt, op in ((0, oa), (1, ob)):
                    osb = recp.tile([128, LQ], F16, tag=f"osb{tt}",
                                    name=f"osb{tt}_{j}")
                    nc.vector.tensor_copy(osb[0:D + 1, :], op[0:D + 1, :])
                    norm_state[(j, tt)] = osb

            def emit_recip(j, tt):
                # transpose Z (row 64) so lq sits on partitions, then a
                # free-size-4 reciprocal, then transpose back
                osb = norm_state[(j, tt)]
                ztp = pssm.tile([128, NL, 128], F16, tag="sm",
                                name=f"ztp{tt}_{j}")
                for b in range(NL):
                    nc.tensor.transpose(ztp[:, b, :], osb[:, ts(b, 128)],
                                        idt16)
                with nc.allow_low_precision(reason="fp16 softmax recip"):
                    nc.vector.reciprocal(rz[tt][:, :, 64:65],
                                         ztp[:, :, 64:65])
                rbt = pssm.tile([128, NL, 128], F16, tag="sm",
                                name=f"rbt{tt}_{j}")
                for b in range(NL):
                    nc.tensor.transpose(rbt[:, b, :], rz[tt][:, b, :], idt16)
                recs = recp.tile([65, NL, 128], F16, tag=f"rec{tt}",
                                 name=f"recs{tt}_{j}")
                nc.vector.tensor_copy(recs[64:65, :, :], rbt[64:65, :, :])
                norm_state[(j, tt, "rec")] = recs

            def emit_norm_finish(j, tt):
                osb = norm_state.pop((j, tt))
                recs = norm_state.pop((j, tt, "rec"))
                rbp = pssm.tile([128, LQ], F32, tag="sm")
                nc.tensor.matmul(rbp[0:D, :], ones16[D:D + 1, :],
                                 recs[64:65, :, :], start=True, stop=True)
                rbs = recp.tile([D, LQ], F16, tag="rbs")
                nc.vector.tensor_copy(rbs, rbp[0:D, :])
                if tt == 0:
                    nc.vector.tensor_mul(obig[0:D, j, :], osb[0:D, :], rbs)
                else:
                    ofin = recp.tile([D, LQ], F16, tag="ofin")
                    nc.vector.tensor_mul(ofin, osb[0:D, :], rbs)
                    nc.gpsimd.dma_start(obig[64:128, j, :], ofin)

            for j in range(PAIRS):
                oa = psacc.tile([128, LQ], F32, tag="acc")
                ob = psacc.tile([128, LQ], F32, tag="acc")

                def emit_pv(c, pt):
                    nc.tensor.matmul(oa[0:D + 1, :], vaug[:, c, :],
                                     pt[:, 0, :], start=(c == 0),
                                     stop=(c == KCH - 1))
                    nc.tensor.matmul(ob[0:D + 1, :], vaug[:, c, :],
                                     pt[:, 1, :], start=(c == 0),
                                     stop=(c == KCH - 1))

                prev = None
                for c in range(KCH):
                    pe_mask = c in PE_SET
                    st = psst.tile([128, 2, LQ], F32, tag="st")
                    for tt, kt in ((0, ktop), (1, kbot)):
                        nc.tensor.matmul(st[:, tt, :], kt[:, ts(c, 128)],
                                         qrot[j], start=True,
                                         stop=not pe_mask)
                        if pe_mask:
                            nc.tensor.matmul(st[:, tt, :], idt16,
                                             mtn[:, c, :], start=False,
                                             stop=True)
                    pt = ptp.tile([128, 2, LQ], F16, tag="p")
                    nc.scalar.activation(pt, st,
                                         mybir.ActivationFunctionType.Exp)
                    if not pe_mask:
                        eng = nc.gpsimd if c in GP_SET else nc.vector
                        eng.tensor_tensor(out=pt, in0=pt,
                                          in1=mt[:, c, :, :],
                                          op=mybir.AluOpType.mult)
                    if debug and j == 0 and c == 0:
                        nc.sync.dma_start(dbg["d_pt0"].ap(), pt)
                    if debug and j == 0 and c == KCH - 1:
                        nc.sync.dma_start(dbg["d_pt15"].ap(), pt)
                    # PV lags one chunk so the PE never waits on exp+mask
                    if prev is not None:
                        emit_pv(c - 1, prev)
                    prev = pt
                    if j > 0:
                        if c == 2:
                            emit_recip(j - 1, 0)
                        elif c == 7:
                            emit_recip(j - 1, 1)
                        elif c == 8:
                            emit_norm_finish(j - 1, 0)
                        elif c == 12:
                            emit_norm_finish(j - 1, 1)
                    if c == 3 and j + 1 < PAIRS:
                        emit_qproj_slice(j + 1, 0, 3)
                    elif c == 4 and j + 1 < PAIRS:
                        emit_qproj_slice(j + 1, 3, 6)
                    elif c == 5 and j + 1 < PAIRS:
                        emit_qproj_slice(j + 1, 6, FCH)
                        emit_qproj_bias(j + 1)
                    elif c == 6 and j + 1 < PAIRS:
                        emit_rope_finish(j + 1)
                    elif c == 13 and j + 2 < PAIRS:
                        prefetch_wq(j + 2)
                    elif c == 14 and j < FCH:
                        # prefetch one wo block per pair into wo_sb
                        nc.sync.dma_start(wo_sb[:, j, :, :], t["wo"].ap()[j])
                emit_pv(KCH - 1, prev)
                if debug and j == 0:
                    dcp_a = yout.tile([128, LQ], F32, tag="y")
                    nc.vector.tensor_copy(dcp_a, oa)
                    nc.sync.dma_start(dbg["d_oa0"].ap(), dcp_a)
                    dcp_b = yout.tile([128, LQ], F32, tag="y")
                    nc.vector.tensor_copy(dcp_b, ob)
                    nc.sync.dma_start(dbg["d_ob0"].ap(), dcp_b)
                emit_evacuate(j, oa, ob)

            # finish last pair's normalization
            emit_recip(PAIRS - 1, 0)
            emit_recip(PAIRS - 1, 1)
            emit_norm_finish(PAIRS - 1, 0)
            emit_norm_finish(PAIRS - 1, 1)

            if debug:
                nc.sync.dma_start(dbg["d_ktop"].ap(), ktop)
                nc.sync.dma_start(dbg["d_kvraw"].ap(), kvraw)
                nc.sync.dma_start(dbg["d_obig"].ap(), obig)
                nc.sync.dma_start(dbg["d_vaug"].ap(), vaug)

            # ================= phase D: output projection =================
            for fb in range(FCH):
                psy = psacc.tile([128, LQ], F32, tag="acc")
                for j in range(FCH):
                    nc.tensor.matmul(psy, wo_sb[:, fb, j, :], obig[:, j, :],
                                     start=(j == 0), stop=(j == FCH - 1))
                ysb = yout.tile([128, LQ], F16, tag="y")
                nc.vector.tensor_scalar_add(ysb, psy, bo_sb[:, fb:fb + 1])
                out_eng = (nc.sync, nc.scalar, nc.gpsimd)[fb % 3]
                out_eng.dma_start(yT.ap()[ts(fb, 128), :], ysb)

    nc.compile()
    return nc


def _tables():
    """RoPE tables in halves-permuted basis: rows i (even-half) hold +sin,
    rows 32+i (odd-half) hold -sin (for the tmp-then-swap formulation)."""
    inv_freq = 1.0 / (10000.0 ** (np.arange(0, D, 2, dtype=np.float64) / D))
    ang = np.outer(inv_freq, np.arange(L, dtype=np.float64))  # [32, L]
    cos = np.cos(ang).astype(np.float32)
    sin = np.sin(ang).astype(np.float32)
    cos64 = np.concatenate([cos, cos], axis=0)                # [64, L]
    sin_sgn = np.concatenate([sin, -sin], axis=0)             # [64, L]
    return cos64, sin_sgn


def _prep_weights(Wq, bq, Wk, bk, Wv, bv, Wo, bo):
    perm = np.concatenate([np.arange(0, D, 2), np.arange(1, D, 2)])
    WqP = np.asarray(Wq, dtype=np.float32)[:, :, perm].reshape(F, H * D)
    bqP = np.asarray(bq, dtype=np.float32)[:, perm].reshape(H * D)
    WkP = np.asarray(Wk, dtype=np.float32)[:, perm]
    bkP = np.asarray(bk, dtype=np.float32)[perm]
    Wkv = np.concatenate([WkP, np.asarray(Wv, dtype=np.float32)], axis=1)
    bkv = np.concatenate([bkP, np.asarray(bv, dtype=np.float32)])
    WoR = np.asarray(Wo, dtype=np.float32).reshape(H * D, F)
    bo_ = np.asarray(bo, dtype=np.float32)

    wq_pre = np.ascontiguousarray(
        WqP.reshape(FCH, 128, FCH, 128).transpose(2, 1, 0, 3)
    ).astype(np.float16)
    wkv_pre = np.ascontiguousarray(
        Wkv.reshape(FCH, 128, 128).transpose(1, 0, 2)).astype(np.float16)
    wo_pre = np.ascontiguousarray(
        WoR.reshape(FCH, 128, FCH, 128).transpose(2, 1, 0, 3)
    ).astype(np.float16)
    bqbo = np.ascontiguousarray(np.concatenate(
        [bqP.reshape(FCH, 128).T, bo_.reshape(FCH, 128).T], axis=1))
    return wq_pre, wkv_pre, wo_pre, bqbo, bkv


def kernel(inputs_q, inputs_kv, mask, Wq, bq, Wk, bk, Wv, bv, Wo, bo):
    if "nc" not in _CACHED:
        _CACHED["nc"] = build_nc(debug=DEBUG)
    nc = _CACHED["nc"]

    wq_pre, wkv_pre, wo_pre, bqbo, bkv = _prep_weights(
        Wq, bq, Wk, bk, Wv, bv, Wo, bo)

    cos64, sin_sgn = _tables()
    scale = 1.0 / np.sqrt(np.float32(D))
    cksk = np.ascontiguousarray(
        np.concatenate([cos64, sin_sgn], axis=1)).astype(np.float16)
    cosq_full = np.tile(cos64 * scale, (2, 1))         # [128, L]
    sinq_full = np.tile(sin_sgn * scale, (2, 1))

    xq = np.asarray(inputs_q, dtype=np.float32)
    xkv = np.asarray(inputs_kv, dtype=np.float32)
    mk = np.asarray(mask)

    in_maps = []
    for core in range(NCORES):
        b = core // 4
        qs = (core % 4) * LQ
        xq_t = np.ascontiguousarray(
            xq[b, qs:qs + LQ, :].T.reshape(FCH, 128, LQ).transpose(1, 0, 2)
        ).astype(np.float16)
        xkv_t = np.ascontiguousarray(
            xkv[b].T.reshape(FCH, 128, NL, LQ).transpose(2, 1, 0, 3)
        ).astype(np.float16)
        mask_1 = mk[b, 0, qs:qs + LQ, :].T.reshape(KCH, 128, LQ)
        mask_t = np.ascontiguousarray(np.broadcast_to(
            mask_1[:, :, None, :], (KCH, 128, 2, LQ)).transpose(1, 0, 2, 3)
        ).astype(np.float16)
        maskn = np.ascontiguousarray(
            ((mask_1.astype(np.float32) - 1.0) * 30.0).transpose(1, 0, 2)
        ).astype(np.float16)
        in_maps.append({
            "xq_t": xq_t,
            "xkv_t": xkv_t,
            "mask_t": mask_t,
            "maskn": maskn,
            "wq": wq_pre,
            "wkv": wkv_pre,
            "wo": wo_pre,
            "bqbo": bqbo,
            "bkv": bkv,
            "cosq": np.ascontiguousarray(cosq_full[:, qs:qs + LQ]),
            "sinq": np.ascontiguousarray(sinq_full[:, qs:qs + LQ]),
            "cksk": cksk,
        })

    res = bass_utils.run_bass_kernel_spmd(nc, in_maps,
                                          core_ids=list(range(NCORES)))
    _CACHED["last_results"] = res
    _CACHED["last_maps"] = in_maps

    out = np.empty((B, L, F), dtype=np.float32)
    for core in range(NCORES):
        b = core // 4
        qs = (core % 4) * LQ
        out[b, qs:qs + LQ, :] = res.results[core]["yT"].T.astype(np.float32)
    return out
